# revision 8
# baseline (speedup 1.0000x reference)
"""Multi-head attention (B=4, S=2048, D=1024, H=16) on 8 TRN2 NeuronCores.

Sharding: core c -> (batch b = c//2, head-group g = c%2 of 8 heads).
Data parallel over batch, tensor parallel over heads; each core computes
its group's QKV projection slices, causal attention for its 8 heads, and
the partial output projection. Host sums the two per-batch partials
(the tensor-parallel unshard) and adds the V-bias epilogue.

On-device layout is "features on partitions": x, Q, K arrive/stay
transposed [feat, seq]; attention scores are computed directly in
transposed form S.T[k, q] so the exp'd probabilities feed the PV matmul
without any on-chip transpose. The softmax denominator rides the PV
matmul as an appended ones-column of V; normalization is a K=1
broadcast matmul + DVE multiply. All matmuls run in float32r.
"""

import os
import numpy as np

B, S, D, H = 4, 2048, 1024, 16
DK = D // H          # 64
HPC = H // 2         # heads per core = 8
GD = HPC * DK        # group feature width = 512
QT = 512             # q-tile width (free dim of S.T chunks)
KTL = 128            # k-tile length (partition dim of S.T chunks)
N_QT = S // QT       # 4
N_KT = S // KTL      # 16
SB = 512             # phase-1 seq block
NEG = np.float32(-1e9)
SCALE = 1.0 / np.sqrt(np.float32(DK))

_cache = {}
last_results = None


def _classify_mask(mask2d):
    """Per (q-tile, k-tile) classification of the [S,S] bool mask.

    Returns (mask_idx[N_QT][N_KT], blocks): idx -2 = fully masked (skip
    block), -1 = unmasked (no bias), >=0 = index into `blocks`, each a
    [KTL, QT] f32 additive bias in S.T (k, q) layout.
    """
    idx = np.full((N_QT, N_KT), -2, dtype=np.int64)
    patterns = {}
    blocks = []
    for qi in range(N_QT):
        for kt in range(N_KT):
            blk = mask2d[qi * QT:(qi + 1) * QT, kt * KTL:(kt + 1) * KTL].T
            if blk.all():
                continue
            if not blk.any():
                idx[qi, kt] = -1
                continue
            key = blk.tobytes()
            if key not in patterns:
                patterns[key] = len(blocks)
                blocks.append(np.where(blk, NEG, np.float32(0.0)))
            idx[qi, kt] = patterns[key]
        if (idx[qi] == -2).all():
            # fully-masked q-row: include everything with full bias so the
            # softmax matches the reference's uniform distribution. (Needs
            # max-subtraction to be exact; this degenerate case does not
            # occur for causal or empty masks.)
            full = np.full((KTL, QT), NEG, dtype=np.float32)
            key = full.tobytes()
            if key not in patterns:
                patterns[key] = len(blocks)
                blocks.append(full)
            idx[qi, :] = patterns[key]
    return idx, blocks


def _build(mask_idx, n_maskb):
    import concourse.bacc as bacc
    import concourse.tile as tile
    import concourse.mybir as mybir

    f32 = mybir.dt.float32
    f32r = mybir.dt.float32r
    Exp = mybir.ActivationFunctionType.Exp
    Ident = mybir.ActivationFunctionType.Identity

    nc = bacc.Bacc(trn_type="TRN2", target_bir_lowering=False, debug=False)
    xT = nc.dram_tensor("xT", [D, S], f32, kind="ExternalInput").ap()
    w_qk = nc.dram_tensor("w_qk", [D, 2 * GD], f32, kind="ExternalInput").ap()
    b_qk = nc.dram_tensor("b_qk", [2 * GD], f32, kind="ExternalInput").ap()
    w_v = nc.dram_tensor("w_v", [D, GD], f32, kind="ExternalInput").ap()
    wo_T = nc.dram_tensor("wo_T", [GD, D], f32, kind="ExternalInput").ap()
    maskb = nc.dram_tensor("maskb", [max(n_maskb, 1), KTL, QT], f32,
                           kind="ExternalInput").ap()
    outT = nc.dram_tensor("outT", [D, S], f32, kind="ExternalOutput").ap()

    ND = D // 128    # 8 contraction chunks
    NM = 2 * GD // 128  # 8 QK feature chunks (0-3 = Q.T, 4-7 = K.T)

    with tile.TileContext(nc) as tc:
        from contextlib import ExitStack
        with ExitStack() as ctx:
            singles = ctx.enter_context(tc.tile_pool(name="singles", bufs=1))
            qkt_pool = ctx.enter_context(tc.tile_pool(name="qkt", bufs=1))
            v_pool = ctx.enter_context(tc.tile_pool(name="vp", bufs=1))
            otg_pool = ctx.enter_context(tc.tile_pool(name="otg", bufs=1))

            # ---- persistent tiles ----
            # Q.T/K.T: 8 chunks [128, S]; chunk m<4 holds Q features for
            # heads (2m, 2m+1), chunk 4+m holds matching K features.
            qkt = [qkt_pool.tile([128, S], f32r, tag=f"qkt{m}", name=f"qkt{m}") for m in range(NM)]
            # V natural layout + ones column: per seq chunk [128, 8, 65].
            v_sb = [v_pool.tile([128, HPC, DK + 1], f32r, tag=f"v{t}", name=f"v{t}") for t in range(N_KT)]
            # normalized attention output, transposed: 4 chunks [128, S]
            otg = [otg_pool.tile([128, S], f32r, tag=f"otg{m}", name=f"otg{m}") for m in range(GD // 128)]
            ones_col = singles.tile([1, DK], f32r)
            nc.vector.memset(ones_col.bitcast(f32), 1.0)
            # biases for Q.T/K.T chunks, per-partition [128, 1]
            bqk_t = singles.tile([128, NM], f32)
            nc.sync.dma_start(out=bqk_t, in_=b_qk.rearrange("(m p) -> p m", p=128))
            # mask bias tiles
            mb_t = []
            if n_maskb:
                for i in range(n_maskb):
                    t = singles.tile([KTL, QT], f32, tag=f"mb{i}", name=f"mb{i}")
                    nc.sync.dma_start(out=t, in_=maskb[i])
                    mb_t.append(t)

            # ============ phase 1a: Q.T / K.T projection ============
            with tc.tile_pool(name="p1w", bufs=1) as p1w, \
                 tc.tile_pool(name="p1x", bufs=9) as p1x, \
                 tc.tile_pool(name="p1ps", bufs=4, space="PSUM") as p1ps:
                wqk_t = [p1w.tile([128, 2 * GD], f32r, tag=f"wqk{k}", name=f"wqk{k}") for k in range(ND)]
                for k in range(ND):
                    nc.sync.dma_start(out=wqk_t[k], in_=w_qk[128 * k:128 * (k + 1)].bitcast(f32r))
                for sb in range(S // SB):
                    xs = [p1x.tile([128, SB], f32r, tag="x", name=f"xs{k}") for k in range(ND)]
                    for k in range(ND):
                        nc.sync.dma_start(
                            out=xs[k],
                            in_=xT[128 * k:128 * (k + 1), SB * sb:SB * (sb + 1)].bitcast(f32r))
                    for m in range(NM):
                        ps = p1ps.tile([128, SB], f32, tag="p1", name="ps_qk")
                        for k in range(ND):
                            nc.tensor.matmul(
                                ps[:], wqk_t[k][:, 128 * m:128 * (m + 1)], xs[k][:],
                                start=(k == 0), stop=(k == ND - 1))
                        nc.scalar.activation(
                            out=qkt[m][:, SB * sb:SB * (sb + 1)], in_=ps[:],
                            func=Ident, bias=bqk_t[:, m:m + 1], scale=1.0)

            # ============ phase 1b: V projection (natural layout) ============
            with tc.tile_pool(name="p1wv", bufs=1) as p1wv, \
                 tc.tile_pool(name="p1xv", bufs=9) as p1xv, \
                 tc.tile_pool(name="p1psv", bufs=4, space="PSUM") as p1psv:
                wv_t = [p1wv.tile([128, GD], f32r, tag=f"wv{k}", name=f"wv{k}") for k in range(ND)]
                for k in range(ND):
                    nc.sync.dma_start(out=wv_t[k], in_=w_v[128 * k:128 * (k + 1)].bitcast(f32r))
                for sb in range(S // SB):
                    xs = [p1xv.tile([128, SB], f32r, tag="xv", name=f"xv{k}") for k in range(ND)]
                    for k in range(ND):
                        nc.sync.dma_start(
                            out=xs[k],
                            in_=xT[128 * k:128 * (k + 1), SB * sb:SB * (sb + 1)].bitcast(f32r))
                    for tt in range(SB // 128):
                        t = sb * (SB // 128) + tt
                        ps = p1psv.tile([128, GD], f32, tag="p1v", name="ps_v")
                        for k in range(ND):
                            nc.tensor.matmul(
                                ps[:], xs[k][:, 128 * tt:128 * (tt + 1)], wv_t[k][:],
                                start=(k == 0), stop=(k == ND - 1))
                        nc.scalar.activation(
                            out=v_sb[t][:, :, 0:DK],
                            in_=ps[:].rearrange("p (h d) -> p h d", h=HPC),
                            func=Ident, scale=1.0)
                        nc.vector.memset(v_sb[t][:, :, DK:DK + 1].bitcast(f32), 1.0)

            # ================= phase 2: attention =================
            with tc.tile_pool(name="st", bufs=2, space="PSUM") as st_pool, \
                 tc.tile_pool(name="ot", bufs=3, space="PSUM") as ot_pool, \
                 tc.tile_pool(name="pt", bufs=3) as pt_pool, \
                 tc.tile_pool(name="rr", bufs=2) as rr_pool:
                for hp in range(HPC // 2):           # head pairs
                    for qi in range(N_QT):
                        kts = [kt for kt in range(N_KT) if mask_idx[qi][kt] != -2]
                        ot_ps = [ot_pool.tile([DK + 1, QT], f32, tag="ot", name="ot_ps") for _ in range(2)]
                        for ki, kt in enumerate(kts):
                            st = st_pool.tile([128, 2 * QT], f32, tag="st", name="st")
                            for h in range(2):       # heads 2hp, 2hp+1
                                lo, hi = 64 * h, 64 * h + 64
                                nc.tensor.matmul(
                                    st[:, QT * h:QT * (h + 1)],
                                    qkt[4 + hp][lo:hi, KTL * kt:KTL * (kt + 1)],
                                    qkt[hp][lo:hi, QT * qi:QT * (qi + 1)],
                                    start=True, stop=True)
                            mi = mask_idx[qi][kt]
                            if mi >= 0:
                                for h in range(2):
                                    nc.vector.tensor_add(
                                        st[:, QT * h:QT * (h + 1)],
                                        st[:, QT * h:QT * (h + 1)], mb_t[mi][:])
                            pt = pt_pool.tile([128, 2 * QT], f32r, tag="pt", name="pt")
                            nc.scalar.activation(out=pt[:], in_=st[:], func=Exp,
                                                 scale=float(SCALE))
                            for h in range(2):
                                nc.tensor.matmul(
                                    ot_ps[h][:],
                                    v_sb[kt][:, 2 * hp + h, :],
                                    pt[:, QT * h:QT * (h + 1)],
                                    start=(ki == 0), stop=(ki == len(kts) - 1))
                        # normalize: r = 1/denominator, broadcast via PE
                        for h in range(2):
                            r_row = rr_pool.tile([1, QT], f32r, tag="rrow", name="r_row")
                            with nc.allow_low_precision(reason="f32r rounding of softmax denominators is within matmul precision"):
                                nc.vector.reciprocal(out=r_row, in_=ot_ps[h][DK:DK + 1, :])
                            rb_ps = ot_pool.tile([DK, QT], f32, tag="rb", name="rb_ps", bufs=1)
                            nc.tensor.matmul(rb_ps[:], ones_col[:], r_row[:],
                                             start=True, stop=True)
                            rb_sb = rr_pool.tile([DK, QT], f32r, tag="rbsb", name="rb_sb")
                            nc.scalar.activation(out=rb_sb[:], in_=rb_ps[:],
                                                 func=Ident, scale=1.0)
                            nc.vector.tensor_mul(
                                otg[hp][64 * h:64 * h + 64, QT * qi:QT * (qi + 1)],
                                ot_ps[h][0:DK, :], rb_sb[:])

            # ================= phase 3: output projection =================
            with tc.tile_pool(name="p3w", bufs=1) as p3w, \
                 tc.tile_pool(name="p3o", bufs=3) as p3o, \
                 tc.tile_pool(name="p3ps", bufs=4, space="PSUM") as p3ps:
                NK3 = GD // 128  # 4
                wo_t = [p3w.tile([128, D], f32r, tag=f"wo{k}", name=f"wo{k}") for k in range(NK3)]
                for k in range(NK3):
                    nc.sync.dma_start(out=wo_t[k], in_=wo_T[128 * k:128 * (k + 1)].bitcast(f32r))
                for nb in range(S // 512):
                    for m in range(D // 128):
                        ps = p3ps.tile([128, 512], f32, tag="p3", name="ps_o")
                        for k in range(NK3):
                            nc.tensor.matmul(
                                ps[:], wo_t[k][:, 128 * m:128 * (m + 1)],
                                otg[k][:, 512 * nb:512 * (nb + 1)],
                                start=(k == 0), stop=(k == NK3 - 1))
                        ob = p3o.tile([128, 512], f32, tag="ob", name="ob")
                        nc.scalar.activation(out=ob[:], in_=ps[:], func=Ident, scale=1.0)
                        nc.sync.dma_start(
                            out=outT[128 * m:128 * (m + 1), 512 * nb:512 * (nb + 1)],
                            in_=ob[:])
    nc.compile()
    return nc


def kernel(encodings_for_qkv, mask, w_qkv, b_qkv, w_o):
    global last_results
    from concourse.bass_utils import run_bass_kernel_spmd

    x = np.ascontiguousarray(np.asarray(encodings_for_qkv, dtype=np.float32))
    mask2d = np.asarray(mask).reshape(S, S).astype(bool)
    w_qkv = np.asarray(w_qkv, dtype=np.float32)
    b_qkv = np.asarray(b_qkv, dtype=np.float32)
    w_o = np.asarray(w_o, dtype=np.float32)

    mask_idx, blocks = _classify_mask(mask2d)
    key = mask_idx.tobytes()
    if key not in _cache:
        _cache[key] = _build(mask_idx.tolist(), len(blocks))
    nc = _cache[key]

    maskb = (np.stack(blocks) if blocks
             else np.zeros((1, KTL, QT), dtype=np.float32))
    wT = np.ascontiguousarray(w_qkv.T)        # [D, 3D]
    woT_full = w_o.T                          # [D(in), D(out)]

    in_maps = []
    for c in range(8):
        b, g = divmod(c, 2)
        cols = slice(GD * g, GD * (g + 1))
        w_qk_g = np.ascontiguousarray(
            np.concatenate([wT[:, 0 * D:][:, cols], wT[:, 1 * D:][:, cols]], axis=1))
        b_qk_g = np.ascontiguousarray(
            np.concatenate([b_qkv[0 * D:1 * D][cols], b_qkv[1 * D:2 * D][cols]]))
        w_v_g = np.ascontiguousarray(wT[:, 2 * D:][:, cols])
        wo_T_g = np.ascontiguousarray(woT_full[cols, :])
        in_maps.append({
            "xT": np.ascontiguousarray(x[b].T),
            "w_qk": w_qk_g, "b_qk": b_qk_g, "w_v": w_v_g,
            "wo_T": wo_T_g, "maskb": maskb,
        })

    trace = bool(int(os.environ.get("KERNEL_PROFILE", "0")))
    res = run_bass_kernel_spmd(nc, in_maps, core_ids=list(range(8)),
                               trace=trace,
                               trace_cores=list(range(8)) if trace else None)
    last_results = res

    out = np.empty((B, S, D), dtype=np.float32)
    for b in range(B):
        acc = res.results[2 * b]["outT"] + res.results[2 * b + 1]["outT"]
        out[b] = acc.T
    # V-bias epilogue: softmax rows sum to 1, so the V bias contributes a
    # constant (b_v @ w_o.T) to every sequence position.
    out += (b_qkv[2 * D:] @ woT_full).reshape(1, 1, D)
    return out
